# revision 21
# baseline (speedup 1.0000x reference)
"""ConvEncoder kernel for 8 TRN2 NeuronCores.

Computes: emb = emb_table[x]; windows = im2col(pad(emb), WIN=5);
y = gelu(windows @ W.T + b), for x (16, 2048) int32 ids.

Sharding: data-parallel over batch — 2 batches per core × 8 cores.
Per-core pipeline (all on device):
  0. explicit load_library(mlp) so the Q7 gather-ucode overlay fetch
     (~10us) overlaps the prologue and index load
  1. dma_gather of bf16 PAIR rows (table viewed as (V/2, 256) bf16;
     idx = id//2 fits int16), chunked so compute trails the Q7
     descriptor-generation spine
  2. parity patch: DVE copy_predicated moves odd-row half over even half
  3. TensorE 128x128 bf16 transposes -> embT (EMB on partitions)
  4. 5 shifted bf16 matmuls per 512-token span, accumulated in PSUM
  5. ScalarE exact-GELU(+bias) PSUM->SBUF, DMA out
"""

import numpy as np

import concourse.bass as bass
import concourse.mybir as mybir
from concourse import bacc
from concourse import library_config
from concourse.bass_utils import run_bass_kernel_spmd
from concourse.masks import make_identity
from concourse.tile import TileContext

B, S, EMB, WIN, OUT, VOCAB = 16, 2048, 128, 5, 128, 50257
NCORES = 8
BPC = B // NCORES              # batches per core
T = BPC * S                    # tokens per core (4096)
NTILE = T // 128               # 128-token tiles per core (32)
TPB = S // 128                 # tiles per batch (16)
SPAN = 512                     # tokens per matmul psum tile
NSPAN = T // SPAN              # psum tiles per core (8)
SPB = S // SPAN                # spans per batch (4)
HALO = WIN // 2                # 2
EC = S + 2 * HALO              # embT columns per batch (2052)
VP = (VOCAB + 1) // 2          # pair-table rows (25129)
D2 = 2 * EMB                   # pair width (256 elems)
NG = 4                         # gather chunks
TPG = T // NG                  # tokens per gather chunk
CPG = NTILE // NG              # col-tiles per gather chunk

MM_DT = mybir.dt.bfloat16

_cache = {}


def _emit_span(nc, wt_sb, embT, b_sb, out, workp, mpsump, j):
    bb, ts0 = j // SPB, (j % SPB) * SPAN
    ps = mpsump.tile([128, SPAN], mybir.dt.float32, space="PSUM", name=f"ps{j}", tag="ps")
    for k in range(WIN):
        nc.tensor.matmul(
            out=ps[:],
            lhsT=wt_sb[:, k * OUT : (k + 1) * OUT],
            rhs=embT[:, bb * EC + ts0 + k : bb * EC + ts0 + k + SPAN],
            start=(k == 0),
            stop=(k == WIN - 1),
        )
    ao = workp.tile([128, SPAN], mybir.dt.float32, name=f"ao{j}", tag="ao")
    nc.scalar.activation(
        out=ao[:], in_=ps[:],
        func=mybir.ActivationFunctionType.Gelu,
        bias=b_sb[:, 0:1],
    )
    nc.sync.dma_start(out=out[:, j * SPAN : (j + 1) * SPAN], in_=ao[:])


def _build():
    nc = bacc.Bacc("TRN2", target_bir_lowering=False, debug=False)
    xg = nc.declare_dram_parameter("xg", [128, T // 16], mybir.dt.int16, isOutput=False)
    so = nc.declare_dram_parameter("so", [128, NTILE], mybir.dt.uint8, isOutput=False)
    tbl = nc.declare_dram_parameter("tbl", [VP, D2], MM_DT, isOutput=False)
    wt = nc.declare_dram_parameter("wt", [128, WIN * OUT], mybir.dt.float32, isOutput=False)
    bv = nc.declare_dram_parameter("bias", [128, 1], mybir.dt.float32, isOutput=False)
    out = nc.declare_dram_parameter("out", [128, T], mybir.dt.float32, isOutput=True)

    # span j's matmuls need embT through token tile 4j+4 (right halo),
    # except batch-end spans (zero-padded)
    span_after_tile = {}
    for j in range(NSPAN):
        need = min(4 * j + 4, ((j // SPB) + 1) * TPB - 1)
        span_after_tile.setdefault(need, []).append(j)

    # prefetch the gather ucode overlay before Tile's prologue so the
    # ~10us Q7 library load overlaps the prologue + index load
    nc.gpsimd.load_library(library_config.mlp)

    with TileContext(nc) as tc:
        with (
            tc.tile_pool(name="const", bufs=1) as constp,
            tc.tile_pool(name="work", bufs=3) as workp,
            tc.tile_pool(name="tpsum", bufs=4, space="PSUM") as tpsump,
            tc.tile_pool(name="mpsum", bufs=3, space="PSUM") as mpsump,
        ):
            idxall = constp.tile([128, T // 16], mybir.dt.int16)
            nc.sync.dma_start(out=idxall[:], in_=xg[:])

            gbp = constp.tile([128, NTILE, D2], MM_DT)
            CIG = TPG // 16  # idx columns per chunk
            for g in range(NG):
                nc.gpsimd.dma_gather(
                    out_ap=gbp[:, g * CPG : (g + 1) * CPG, :],
                    in_ap=tbl[:],
                    idxs_ap=idxall[:, g * CIG : (g + 1) * CIG],
                    num_idxs=TPG,
                    num_idxs_reg=TPG,
                    elem_size=D2,
                )

            sel = constp.tile([128, NTILE], mybir.dt.uint8)
            nc.sync.dma_start(out=sel[:], in_=so[:])
            wt_f32 = constp.tile([128, WIN * OUT], mybir.dt.float32)
            nc.sync.dma_start(out=wt_f32[:], in_=wt[:])
            wt_sb = constp.tile([128, WIN * OUT], MM_DT)
            nc.vector.tensor_copy(out=wt_sb[:], in_=wt_f32[:])
            b_sb = constp.tile([128, 1], mybir.dt.float32)
            nc.sync.dma_start(out=b_sb[:], in_=bv[:])
            ident = constp.tile([128, 128], MM_DT)
            make_identity(nc, ident[:])

            # emb transposed: embT[e, bb*EC + 2 + t] = emb[bb, t, e]; halo cols zero
            embT = constp.tile([128, BPC * EC], MM_DT)
            for bb in range(BPC):
                nc.vector.memset(embT[:, bb * EC : bb * EC + HALO], 0.0)
                nc.vector.memset(embT[:, bb * EC + HALO + S : (bb + 1) * EC], 0.0)

            for c in range(NTILE):
                # odd-id tokens: take the second half of the pair row
                nc.vector.copy_predicated(
                    out=gbp[:, c, 0:EMB],
                    mask=sel[:, c : c + 1].to_broadcast([128, EMB]),
                    data=gbp[:, c, EMB:D2],
                )
                pt = tpsump.tile([128, 128], MM_DT, space="PSUM", name=f"pt{c}", tag="pt")
                nc.tensor.transpose(out=pt[:], in_=gbp[:, c, 0:EMB], identity=ident[:])
                bb, tl = c // TPB, (c % TPB) * 128
                nc.vector.tensor_copy(
                    out=embT[:, bb * EC + HALO + tl : bb * EC + HALO + tl + 128],
                    in_=pt[:],
                )
                for j in span_after_tile.get(c, []):
                    _emit_span(nc, wt_sb, embT, b_sb, out, workp, mpsump, j)

    nc.compile()
    return nc


def _prep_inputs(x, emb_table, W, b):
    import ml_dtypes

    x = np.asarray(x).astype(np.int32)
    emb_table = np.asarray(emb_table, dtype=np.float32)
    W = np.asarray(W, dtype=np.float32)
    b = np.asarray(b, dtype=np.float32)
    tblp = np.zeros((VP, D2), ml_dtypes.bfloat16)
    tblp.reshape(-1)[: VOCAB * EMB] = emb_table.reshape(-1).astype(ml_dtypes.bfloat16)
    wt = np.ascontiguousarray(
        W.reshape(OUT, WIN, EMB).transpose(2, 1, 0).reshape(EMB, WIN * OUT)
    )
    bias = np.ascontiguousarray(b.reshape(128, 1))
    in_maps = []
    for core in range(NCORES):
        flat = x[core * BPC : (core + 1) * BPC].reshape(-1)
        idx16 = (flat // 2).astype(np.int16)
        xgc = np.concatenate([
            np.tile(idx16[g * TPG : (g + 1) * TPG].reshape(TPG // 16, 16).T, (8, 1))
            for g in range(NG)
        ], axis=1)
        soc = np.ascontiguousarray((flat & 1).astype(np.uint8).reshape(NTILE, 128).T)
        in_maps.append({
            "xg": np.ascontiguousarray(xgc), "so": soc, "tbl": tblp,
            "wt": wt, "bias": bias,
        })
    return in_maps


def kernel(x, emb_table, W, b, _trace=False):
    if "nc" not in _cache:
        _cache["nc"] = _build()
    nc = _cache["nc"]
    in_maps = _prep_inputs(x, emb_table, W, b)
    res = run_bass_kernel_spmd(nc, in_maps, core_ids=list(range(NCORES)), trace=_trace)
    _cache["last_result"] = res
    outs = []
    for core in range(NCORES):
        oc = res.results[core]["out"]          # (128, T)
        outs.append(oc.T.reshape(BPC, S, OUT))
    return np.concatenate(outs, axis=0)


# revision 22
# speedup vs baseline: 1.1930x; 1.1930x over previous
"""ConvEncoder kernel for 8 TRN2 NeuronCores.

Computes: emb = emb_table[x]; windows = im2col(pad(emb), WIN=5);
y = gelu(windows @ W.T + b), for x (16, 2048) int32 ids.

Sharding: data-parallel over batch — 2 batches per core × 8 cores.
Per-core pipeline (all on device):
  1. per-128-token indirect-DMA gathers (one int32 row index per
     partition, f32 rows cast to bf16 in-flight by the SWDGE)
  2. TensorE 128x128 bf16 transposes -> embT (EMB on partitions)
  3. 5 shifted bf16 matmuls per 512-token span, accumulated in PSUM
  4. ScalarE exact-GELU(+bias) PSUM->SBUF, DMA out
Gathers, transposes and matmuls are interleaved in program order so
compute trails the gather spine tile-by-tile and the PE stays warm.
The Q7 descriptor-generation rate (~1.1us per 128-row gather) is the
hard serial bottleneck on this hardware; everything else hides under it.
"""

import numpy as np

import concourse.bass as bass
import concourse.mybir as mybir
from concourse import bacc
from concourse.bass import IndirectOffsetOnAxis
from concourse.bass_utils import run_bass_kernel_spmd
from concourse.tile import TileContext

B, S, EMB, WIN, OUT, VOCAB = 16, 2048, 128, 5, 128, 50257
NCORES = 8
BPC = B // NCORES              # batches per core
T = BPC * S                    # tokens per core (4096)
NTILE = T // 128               # 128-token tiles per core (32)
TPB = S // 128                 # tiles per batch (16)
SPAN = 512                     # tokens per matmul psum tile
NSPAN = T // SPAN              # psum tiles per core (8)
SPB = S // SPAN                # spans per batch (4)
HALO = WIN // 2                # 2
EC = S + 2 * HALO              # embT columns per batch (2052)

MM_DT = mybir.dt.bfloat16

_cache = {}


def _emit_span(nc, wt_sb, embT, b_sb, out, workp, mpsump, j, half=None):
    """Emit one 512-token span (or one 256-token half if half is 0/1)."""
    bb, ts0 = j // SPB, (j % SPB) * SPAN
    if half is None:
        off, width = 0, SPAN
    else:
        off, width = half * (SPAN // 2), SPAN // 2
    ps = mpsump.tile([128, SPAN // 2 if half is not None else SPAN], mybir.dt.float32,
                     space="PSUM", name=f"ps{j}_{half}", tag="ps")
    for k in range(WIN):
        nc.tensor.matmul(
            out=ps[:],
            lhsT=wt_sb[:, k * OUT : (k + 1) * OUT],
            rhs=embT[:, bb * EC + ts0 + off + k : bb * EC + ts0 + off + k + width],
            start=(k == 0),
            stop=(k == WIN - 1),
        )
    ao = workp.tile([128, width], mybir.dt.float32, name=f"ao{j}_{half}", tag="ao")
    nc.scalar.activation(
        out=ao[:], in_=ps[:],
        func=mybir.ActivationFunctionType.Gelu,
        bias=b_sb[:, 0:1],
    )
    nc.sync.dma_start(out=out[:, j * SPAN + off : j * SPAN + off + width], in_=ao[:])


def _build():
    nc = bacc.Bacc("TRN2", target_bir_lowering=False, debug=False)
    xi = nc.declare_dram_parameter("xi", [128, NTILE], mybir.dt.int32, isOutput=False)
    tbl = nc.declare_dram_parameter("tbl", [VOCAB, EMB], mybir.dt.float32, isOutput=False)
    wt = nc.declare_dram_parameter("wt", [128, WIN * OUT], mybir.dt.float32, isOutput=False)
    bv = nc.declare_dram_parameter("bias", [128, 1], mybir.dt.float32, isOutput=False)
    idm = nc.declare_dram_parameter("idm", [128, 128], mybir.dt.uint16, isOutput=False)
    out = nc.declare_dram_parameter("out", [128, T], mybir.dt.float32, isOutput=True)

    # span j's matmuls need embT through token tile 4j+4 (right halo),
    # except batch-end spans (zero-padded)
    span_after_tile = {}
    for j in range(NSPAN):
        need = min(4 * j + 4, ((j // SPB) + 1) * TPB - 1)
        span_after_tile.setdefault(need, []).append(j)

    with TileContext(nc) as tc:
        with (
            tc.tile_pool(name="const", bufs=1) as constp,
            tc.tile_pool(name="work", bufs=4) as workp,
            tc.tile_pool(name="tpsum", bufs=4, space="PSUM") as tpsump,
            tc.tile_pool(name="mpsum", bufs=3, space="PSUM") as mpsump,
        ):
            idx_sb = constp.tile([128, NTILE], mybir.dt.int32)
            nc.sync.dma_start(out=idx_sb[:], in_=xi[:])
            idxcs = []
            for c in range(NTILE):
                idxc = constp.tile([128, 1], mybir.dt.int32, name=f"idxc{c}", tag="idxc", bufs=NTILE)
                nc.vector.tensor_copy(out=idxc[:], in_=idx_sb[:, c : c + 1])
                idxcs.append(idxc)

            gb = constp.tile([128, NTILE, EMB], MM_DT)
            embT = constp.tile([128, BPC * EC], MM_DT)
            for bb in range(BPC):
                nc.vector.memset(embT[:, bb * EC : bb * EC + HALO], 0.0)
                nc.vector.memset(embT[:, bb * EC + HALO + S : (bb + 1) * EC], 0.0)

            sel_loads_done = False

            for c in range(NTILE):
                nc.gpsimd.indirect_dma_start(
                    out=gb[:, c, :],
                    out_offset=None,
                    in_=tbl[:],
                    in_offset=IndirectOffsetOnAxis(ap=idxcs[c][:], axis=0),
                )
                if not sel_loads_done:
                    # constants loaded once, right after the first gather is
                    # issued so they don't delay the gather spine
                    sel_loads_done = True
                    wt_f32 = constp.tile([128, WIN * OUT], mybir.dt.float32)
                    nc.sync.dma_start(out=wt_f32[:], in_=wt[:])
                    wt_sb = constp.tile([128, WIN * OUT], MM_DT)
                    nc.vector.tensor_copy(out=wt_sb[:], in_=wt_f32[:])
                    b_sb = constp.tile([128, 1], mybir.dt.float32)
                    nc.sync.dma_start(out=b_sb[:], in_=bv[:])
                    ident = constp.tile([128, 128], MM_DT)
                    nc.sync.dma_start(out=ident[:].bitcast(mybir.dt.uint16), in_=idm[:])

                pt = tpsump.tile([128, 128], MM_DT, space="PSUM", name=f"pt{c}", tag="pt")
                nc.tensor.transpose(out=pt[:], in_=gb[:, c, :], identity=ident[:])
                bb, tl = c // TPB, (c % TPB) * 128
                nc.vector.tensor_copy(
                    out=embT[:, bb * EC + HALO + tl : bb * EC + HALO + tl + 128],
                    in_=pt[:],
                )
                for j in span_after_tile.get(c, []):
                    if c == NTILE - 1:
                        # final span in two halves so the activation + store
                        # of the first half overlaps the second half's matmuls
                        _emit_span(nc, wt_sb, embT, b_sb, out, workp, mpsump, j, half=0)
                        _emit_span(nc, wt_sb, embT, b_sb, out, workp, mpsump, j, half=1)
                    else:
                        _emit_span(nc, wt_sb, embT, b_sb, out, workp, mpsump, j)

    nc.compile()
    return nc


def _prep_inputs(x, emb_table, W, b):
    import ml_dtypes

    x = np.asarray(x).astype(np.int32)
    emb_table = np.ascontiguousarray(np.asarray(emb_table, dtype=np.float32))
    W = np.asarray(W, dtype=np.float32)
    b = np.asarray(b, dtype=np.float32)
    wt = np.ascontiguousarray(
        W.reshape(OUT, WIN, EMB).transpose(2, 1, 0).reshape(EMB, WIN * OUT)
    )
    bias = np.ascontiguousarray(b.reshape(128, 1))
    idm = np.eye(128, dtype=ml_dtypes.bfloat16).view(np.uint16)
    in_maps = []
    for core in range(NCORES):
        flat = x[core * BPC : (core + 1) * BPC].reshape(-1)
        xic = np.ascontiguousarray(flat.reshape(NTILE, 128).T)
        in_maps.append({"xi": xic, "tbl": emb_table, "wt": wt, "bias": bias, "idm": idm})
    return in_maps


def kernel(x, emb_table, W, b, _trace=False):
    if "nc" not in _cache:
        _cache["nc"] = _build()
    nc = _cache["nc"]
    in_maps = _prep_inputs(x, emb_table, W, b)
    res = run_bass_kernel_spmd(nc, in_maps, core_ids=list(range(NCORES)), trace=_trace)
    _cache["last_result"] = res
    outs = []
    for core in range(NCORES):
        oc = res.results[core]["out"]          # (128, T)
        outs.append(oc.T.reshape(BPC, S, OUT))
    return np.concatenate(outs, axis=0)
